# revision 8
# baseline (speedup 1.0000x reference)
"""Locally-connected 2D conv (unshared weights), VALID, stride 2 — Trainium2 Bass kernel.

Problem (hardcoded):
  x:       (16, 32, 113, 113) f32
  weights: (56, 56, 32, 3, 3, 64) f32   (H_out, W_out, C_in, kh, kw, C_out)
  bias:    (56, 56, 64) f32
  out:     (16, 64, 56, 56) f32
  out[b,o,u,v] = sum_{c,q,r} x[b,c,2u+q,2v+r] * weights[u,v,c,q,r,o] + bias[u,v,o]

Sharding: H_out split across 8 cores (7 output rows each).

Design notes (v3):
- The weight tensor is touched exactly once, so the kernel is pure HBM-traffic
  bound. Per-SDMA-engine throughput measures ~15 GB/s here regardless of
  packet size or queue mix (port shared with the sibling NeuronCore; all 8
  cores stream concurrently), i.e. ~240 GB/s/core. So: minimize bytes, keep
  all 16 engines fed the whole span.
- Weights/x stream as bf16 (rel err ~2.5e-3 vs the 2e-2 gate), output in
  bf16 too: 14.45 (w) + 2.58 (x) + 0.8 (out) MB per core.
- Matmul: x window columns are the *stationary* operand (16-col LDWEIGHTS
  ~13 ns) and weight blocks are the *moving* operand at 1 col/cycle@2.4GHz.
  Contraction k = (r, c) on 96 partitions; q accumulates in PSUM. x is packed
  host-side so partition p = r*32+c holds x[b, c, row, 2v+r] (1.49x
  replication, vs 2.6x for the (q,c) packing).
- PSUM: out partitions = batch (16). Four PE column groups (tile_position
  (0, 32g)) pack v = 28h+7g+vl into one [128, 448] bank per (u, h). Each
  group needs its own start=True (the has_written clear only covers the
  partitions that matmul writes). vl-outer issue order lets the 4 column
  groups run concurrently in the array.
- Weight DMA in 14 per-(u,h) chunks (1MB each), bufs=8 so prefetch never
  stalls on tile recycling. ALL DMAs ride the single SP HWDGE ring: Tile
  assigns DMA-completion semaphore lanes round-robin across rings, and lane
  thresholds are only race-free if same-lane DMAs complete in tick order —
  guaranteed within one FIFO ring, NOT across two (observed as a rare NaN:
  an MM chunk consuming its weight tile ~5us before the DMA landed). One
  ring costs nothing: descriptors still spray across all 16 SDMA engines.
  Bias is added on the host after the gather.
"""

import numpy as np

B = 16
C_IN = 32
C_OUT = 64
H_OUT = 56
W_OUT = 56
KK = 3
STRIDE = 2
H_IN = 113

N_CORES = 8
U_PER = H_OUT // N_CORES          # 7 output rows per core
ROWS_IN = (U_PER - 1) * STRIDE + KK  # 15 input rows per core
KPART = KK * C_IN                 # 96 contraction partitions (r, c)
G = 4                             # PE column groups
VL = 7                            # v per group per half
NH = 2                            # halves per u
XFREE = ROWS_IN * B * W_OUT       # x tile free elems (row, b, v) = 13440
WFREE_H = G * VL * KK * C_OUT     # weight free elems per (u, h) = 5376
OFREE_U = NH * VL * C_OUT         # output free elems per (u, g) = 896

_CACHE = {}


def _build():
    import concourse.mybir as mybir
    from concourse import bacc
    from concourse.tile import TileContext

    bf16 = mybir.dt.bfloat16
    f32 = mybir.dt.float32
    nc = bacc.Bacc("TRN2", target_bir_lowering=False, debug=False,
                   num_devices=N_CORES)
    # Host-prepacked tensors (see kernel()):
    #   xr[p, (row*16 + b)*56 + v] = x[b, c, 2u0+row, 2v+r],  p = r*32+c
    #   wr[u, h, p, (((g*7+vl)*3+q)*64 + o] = weights[u0+u, 28h+7g+vl, c, q, r, o]
    #   y[g, u, b, (h*7+vl)*64 + o] = out[b, o, u0+u, 28h+7g+vl] (no bias)
    xr_in = nc.dram_tensor("xr", [KPART, XFREE], bf16,
                           kind="ExternalInput").ap()
    wr_in = nc.dram_tensor("wr", [U_PER, NH, KPART, WFREE_H], bf16,
                           kind="ExternalInput").ap()
    y_out = nc.dram_tensor("y", [G, U_PER, B, OFREE_U], bf16,
                           kind="ExternalOutput").ap()

    with TileContext(nc) as tc:
        with tc.tile_pool(name="xpool", bufs=1) as xpool, \
             tc.tile_pool(name="wpool", bufs=8) as wpool, \
             tc.tile_pool(name="opool", bufs=1) as opool, \
             tc.tile_pool(name="pspool", bufs=6, space="PSUM") as pspool:

            # x in 3 row-chunks (5 input rows each), interleaved into the
            # rings exactly where needed (chunk 0 -> u<2, 1 -> u<4, 2 -> u<6)
            # so x never head-of-line-blocks a weight chunk the PE is about
            # to need (PE executes matmuls strictly in order).
            xt = xpool.tile([KPART, XFREE], bf16)
            chunk = 5 * B * W_OUT

            def x_chunk(ci, eng):
                eng.dma_start(out=xt[:, ci * chunk:(ci + 1) * chunk],
                              in_=xr_in[:, ci * chunk:(ci + 1) * chunk])

            x_chunk(0, nc.sync)
            xt4 = xt.rearrange("p (row b v) -> p row b v", row=ROWS_IN, b=B)

            stage = opool.tile([128, U_PER * OFREE_U], bf16)
            st3 = stage.rearrange("p (u x) -> p u x", u=U_PER)

            for u in range(U_PER):
                if u == 1:
                    x_chunk(1, nc.sync)
                elif u == 3:
                    x_chunk(2, nc.sync)
                for h in range(NH):
                    wt = wpool.tile([KPART, WFREE_H], bf16)
                    nc.sync.dma_start(out=wt[:], in_=wr_in[u, h])
                    wt5 = wt.rearrange("p (g vl q o) -> p g vl q o",
                                       g=G, vl=VL, q=KK)

                    ps = pspool.tile([128, VL * C_OUT], f32)
                    ps3 = ps.rearrange("p (vl o) -> p vl o", vl=VL)
                    # vl-outer: the 4 col groups interleave so they run
                    # concurrently in the array. Each group gets its own
                    # start=True (the bank clear only covers the partitions
                    # that matmul writes).
                    for vl in range(VL):
                        for g in range(G):
                            for q in range(KK):
                                lhsT = xt4[:, 2 * u + q, :, 28 * h + 7 * g + vl]
                                rhs = wt5[:, g, vl, q]            # [96, 64]
                                nc.tensor.matmul(
                                    ps3[32 * g:32 * g + 16, vl], lhsT, rhs,
                                    start=(vl == 0 and q == 0),
                                    stop=(vl == VL - 1 and q == KK - 1),
                                    tile_position=(0, 32 * g))
                    # whole-tile drain on the vector engine (f32->bf16);
                    # garbage partitions 16:32 etc. are copied, never DMA'd
                    nc.vector.tensor_scalar_add(
                        st3[:, u, 448 * h:448 * (h + 1)], ps[:, :], 0.0)
                for g in range(G):
                    nc.sync.dma_start(out=y_out[g, u],
                                   in_=st3[32 * g:32 * g + 16, u])

    nc.compile()
    return nc


def _get_nc():
    if "nc" not in _CACHE:
        _CACHE["nc"] = _build()
    return _CACHE["nc"]


def kernel(x, weights, bias, _trace=False, _tmpdir=None):
    import ml_dtypes
    from concourse.bass_utils import run_bass_kernel_spmd

    bf16 = ml_dtypes.bfloat16
    x = np.asarray(x, dtype=np.float32)
    weights = np.asarray(weights, dtype=np.float32)
    bias = np.asarray(bias, dtype=np.float32)

    # wr: (core, u, h, p=(r,c), (g,vl,q,o))
    wb = weights.astype(bf16).reshape(N_CORES, U_PER, NH, G, VL,
                                      C_IN, KK, KK, C_OUT)
    wr_all = np.ascontiguousarray(
        wb.transpose(0, 1, 2, 7, 5, 3, 4, 6, 8)).reshape(
            N_CORES, U_PER, NH, KPART, WFREE_H)

    xb = x.astype(bf16)
    in_maps = []
    for i in range(N_CORES):
        u0 = i * U_PER
        xs = xb[:, :, STRIDE * u0:STRIDE * u0 + ROWS_IN, :]  # (B, C, 15, 113)
        # (r, c, row, b, v): p = r*32+c holds x[b, c, row, 2v+r]
        xq = np.stack([xs[:, :, :, r::2][:, :, :, :W_OUT] for r in range(KK)],
                      axis=0)                                # (r, B, C, 15, 56)
        xr = np.ascontiguousarray(xq.transpose(0, 2, 3, 1, 4)).reshape(
            KPART, XFREE)
        in_maps.append({"xr": xr, "wr": wr_all[i]})

    nc = _get_nc()
    core_ids = list(range(N_CORES))
    res = run_bass_kernel_spmd(nc, in_maps, core_ids, trace=_trace,
                               tmpdir=_tmpdir)
    # y per core: (G, U_PER, B, (h, vl, o)) -> (b, o, core*7+u, 28h+7g+vl)
    ys = np.stack([np.asarray(res.results[i]["y"]) for i in core_ids])
    ys = ys.reshape(N_CORES, G, U_PER, B, NH, VL, C_OUT).astype(np.float32)
    out = np.ascontiguousarray(
        ys.transpose(3, 6, 0, 2, 4, 1, 5)).reshape(B, C_OUT, H_OUT, W_OUT)
    out += bias.transpose(2, 0, 1)[None]
    if _trace:
        _CACHE["last_result"] = res
    return out


# revision 9
# speedup vs baseline: 1.0789x; 1.0789x over previous
"""Locally-connected 2D conv (unshared weights), VALID, stride 2 — Trainium2 Bass kernel.

Problem (hardcoded):
  x:       (16, 32, 113, 113) f32
  weights: (56, 56, 32, 3, 3, 64) f32   (H_out, W_out, C_in, kh, kw, C_out)
  bias:    (56, 56, 64) f32
  out:     (16, 64, 56, 56) f32
  out[b,o,u,v] = sum_{c,q,r} x[b,c,2u+q,2v+r] * weights[u,v,c,q,r,o] + bias[u,v,o]

Sharding: H_out split across 8 cores (7 output rows each).

Design notes (v3):
- The weight tensor is touched exactly once, so the kernel is pure HBM-traffic
  bound. Per-SDMA-engine throughput measures ~15 GB/s here regardless of
  packet size or queue mix (port shared with the sibling NeuronCore; all 8
  cores stream concurrently), i.e. ~240 GB/s/core. So: minimize bytes, keep
  all 16 engines fed the whole span.
- Weights/x stream as bf16 (rel err ~2.5e-3 vs the 2e-2 gate), output in
  bf16 too: 14.45 (w) + 2.58 (x) + 0.8 (out) MB per core.
- Matmul: x window columns are the *stationary* operand (16-col LDWEIGHTS
  ~13 ns) and weight blocks are the *moving* operand at 1 col/cycle@2.4GHz.
  Contraction k = (r, c) on 96 partitions; q accumulates in PSUM. x is packed
  host-side so partition p = r*32+c holds x[b, c, row, 2v+r] (1.49x
  replication, vs 2.6x for the (q,c) packing).
- PSUM: out partitions = batch (16). Four PE column groups (tile_position
  (0, 32g)) pack v = 28h+7g+vl into one [128, 448] bank per (u, h). Each
  group needs its own start=True (the has_written clear only covers the
  partitions that matmul writes). vl-outer issue order lets the 4 column
  groups run concurrently in the array.
- Weight DMA in 14 per-(u,h) chunks (1MB each), bufs=8 so prefetch never
  stalls on tile recycling. DMA rings: weights alone on the SP HWDGE ring;
  x and outputs on the gpsimd SWDGE ring. Rationale: Tile assigns HWDGE
  completion-semaphore lanes round-robin across ALL HWDGE DMAs, and lane
  thresholds are only race-free if same-lane DMAs complete in tick order —
  guaranteed within one FIFO ring, NOT across the two HWDGE rings (observed
  as a rare NaN: an MM chunk consuming its weight tile ~5us before the DMA
  landed). SWDGE uses a separate lane class (DMASW), so it adds a second
  descriptor stream with no aliasing; it also keeps the out-DMAs (which
  wait on drains) from head-of-line-blocking weight prefetch.
  Bias is added on the host after the gather.
"""

import numpy as np

B = 16
C_IN = 32
C_OUT = 64
H_OUT = 56
W_OUT = 56
KK = 3
STRIDE = 2
H_IN = 113

N_CORES = 8
U_PER = H_OUT // N_CORES          # 7 output rows per core
ROWS_IN = (U_PER - 1) * STRIDE + KK  # 15 input rows per core
KPART = KK * C_IN                 # 96 contraction partitions (r, c)
G = 4                             # PE column groups
VL = 7                            # v per group per half
NH = 2                            # halves per u
XFREE = ROWS_IN * B * W_OUT       # x tile free elems (row, b, v) = 13440
WFREE_H = G * VL * KK * C_OUT     # weight free elems per (u, h) = 5376
OFREE_U = NH * VL * C_OUT         # output free elems per (u, g) = 896

_CACHE = {}


def _build():
    import concourse.mybir as mybir
    from concourse import bacc
    from concourse.tile import TileContext

    bf16 = mybir.dt.bfloat16
    f32 = mybir.dt.float32
    nc = bacc.Bacc("TRN2", target_bir_lowering=False, debug=False,
                   num_devices=N_CORES)
    # Host-prepacked tensors (see kernel()):
    #   xr[p, (row*16 + b)*56 + v] = x[b, c, 2u0+row, 2v+r],  p = r*32+c
    #   wr[u, h, p, (((g*7+vl)*3+q)*64 + o] = weights[u0+u, 28h+7g+vl, c, q, r, o]
    #   y[g, u, b, (h*7+vl)*64 + o] = out[b, o, u0+u, 28h+7g+vl] (no bias)
    xr_in = nc.dram_tensor("xr", [KPART, XFREE], bf16,
                           kind="ExternalInput").ap()
    wr_in = nc.dram_tensor("wr", [U_PER, NH, KPART, WFREE_H], bf16,
                           kind="ExternalInput").ap()
    y_out = nc.dram_tensor("y", [G, U_PER, B, OFREE_U], bf16,
                           kind="ExternalOutput").ap()

    with TileContext(nc) as tc:
        with tc.tile_pool(name="xpool", bufs=1) as xpool, \
             tc.tile_pool(name="wpool", bufs=8) as wpool, \
             tc.tile_pool(name="opool", bufs=1) as opool, \
             tc.tile_pool(name="pspool", bufs=6, space="PSUM") as pspool:

            # x in 3 row-chunks (5 input rows each), interleaved into the
            # rings exactly where needed (chunk 0 -> u<2, 1 -> u<4, 2 -> u<6)
            # so x never head-of-line-blocks a weight chunk the PE is about
            # to need (PE executes matmuls strictly in order).
            xt = xpool.tile([KPART, XFREE], bf16)
            chunk = 5 * B * W_OUT

            def x_chunk(ci, eng):
                eng.dma_start(out=xt[:, ci * chunk:(ci + 1) * chunk],
                              in_=xr_in[:, ci * chunk:(ci + 1) * chunk])

            x_chunk(0, nc.gpsimd)
            xt4 = xt.rearrange("p (row b v) -> p row b v", row=ROWS_IN, b=B)

            stage = opool.tile([128, U_PER * OFREE_U], bf16)
            st3 = stage.rearrange("p (u x) -> p u x", u=U_PER)

            for u in range(U_PER):
                if u == 1:
                    x_chunk(1, nc.gpsimd)
                elif u == 3:
                    x_chunk(2, nc.gpsimd)
                for h in range(NH):
                    wt = wpool.tile([KPART, WFREE_H], bf16)
                    nc.sync.dma_start(out=wt[:], in_=wr_in[u, h])
                    wt5 = wt.rearrange("p (g vl q o) -> p g vl q o",
                                       g=G, vl=VL, q=KK)

                    ps = pspool.tile([128, VL * C_OUT], f32)
                    ps3 = ps.rearrange("p (vl o) -> p vl o", vl=VL)
                    # vl-outer: the 4 col groups interleave so they run
                    # concurrently in the array. Each group gets its own
                    # start=True (the bank clear only covers the partitions
                    # that matmul writes).
                    for vl in range(VL):
                        for g in range(G):
                            for q in range(KK):
                                lhsT = xt4[:, 2 * u + q, :, 28 * h + 7 * g + vl]
                                rhs = wt5[:, g, vl, q]            # [96, 64]
                                nc.tensor.matmul(
                                    ps3[32 * g:32 * g + 16, vl], lhsT, rhs,
                                    start=(vl == 0 and q == 0),
                                    stop=(vl == VL - 1 and q == KK - 1),
                                    tile_position=(0, 32 * g))
                    # whole-tile drain on the vector engine (f32->bf16);
                    # garbage partitions 16:32 etc. are copied, never DMA'd
                    nc.vector.tensor_scalar_add(
                        st3[:, u, 448 * h:448 * (h + 1)], ps[:, :], 0.0)
                for g in range(G):
                    nc.gpsimd.dma_start(out=y_out[g, u],
                                        in_=st3[32 * g:32 * g + 16, u])

    nc.compile()
    return nc


def _get_nc():
    if "nc" not in _CACHE:
        _CACHE["nc"] = _build()
    return _CACHE["nc"]


def kernel(x, weights, bias, _trace=False, _tmpdir=None):
    import ml_dtypes
    from concourse.bass_utils import run_bass_kernel_spmd

    bf16 = ml_dtypes.bfloat16
    x = np.asarray(x, dtype=np.float32)
    weights = np.asarray(weights, dtype=np.float32)
    bias = np.asarray(bias, dtype=np.float32)

    # wr: (core, u, h, p=(r,c), (g,vl,q,o))
    wb = weights.astype(bf16).reshape(N_CORES, U_PER, NH, G, VL,
                                      C_IN, KK, KK, C_OUT)
    wr_all = np.ascontiguousarray(
        wb.transpose(0, 1, 2, 7, 5, 3, 4, 6, 8)).reshape(
            N_CORES, U_PER, NH, KPART, WFREE_H)

    xb = x.astype(bf16)
    in_maps = []
    for i in range(N_CORES):
        u0 = i * U_PER
        xs = xb[:, :, STRIDE * u0:STRIDE * u0 + ROWS_IN, :]  # (B, C, 15, 113)
        # (r, c, row, b, v): p = r*32+c holds x[b, c, row, 2v+r]
        xq = np.stack([xs[:, :, :, r::2][:, :, :, :W_OUT] for r in range(KK)],
                      axis=0)                                # (r, B, C, 15, 56)
        xr = np.ascontiguousarray(xq.transpose(0, 2, 3, 1, 4)).reshape(
            KPART, XFREE)
        in_maps.append({"xr": xr, "wr": wr_all[i]})

    nc = _get_nc()
    core_ids = list(range(N_CORES))
    res = run_bass_kernel_spmd(nc, in_maps, core_ids, trace=_trace,
                               tmpdir=_tmpdir)
    # y per core: (G, U_PER, B, (h, vl, o)) -> (b, o, core*7+u, 28h+7g+vl)
    ys = np.stack([np.asarray(res.results[i]["y"]) for i in core_ids])
    ys = ys.reshape(N_CORES, G, U_PER, B, NH, VL, C_OUT).astype(np.float32)
    out = np.ascontiguousarray(
        ys.transpose(3, 6, 0, 2, 4, 1, 5)).reshape(B, C_OUT, H_OUT, W_OUT)
    out += bias.transpose(2, 0, 1)[None]
    if _trace:
        _CACHE["last_result"] = res
    return out


# revision 10
# speedup vs baseline: 1.2444x; 1.1533x over previous
"""Locally-connected 2D conv (unshared weights), VALID, stride 2 — Trainium2 Bass kernel.

Problem (hardcoded):
  x:       (16, 32, 113, 113) f32
  weights: (56, 56, 32, 3, 3, 64) f32   (H_out, W_out, C_in, kh, kw, C_out)
  bias:    (56, 56, 64) f32
  out:     (16, 64, 56, 56) f32
  out[b,o,u,v] = sum_{c,q,r} x[b,c,2u+q,2v+r] * weights[u,v,c,q,r,o] + bias[u,v,o]

Sharding: H_out split across 8 cores (7 output rows each).

Design notes (v3):
- The weight tensor is touched exactly once, so the kernel is pure HBM-traffic
  bound. Per-SDMA-engine throughput measures ~15 GB/s here regardless of
  packet size or queue mix (port shared with the sibling NeuronCore; all 8
  cores stream concurrently), i.e. ~240 GB/s/core. So: minimize bytes, keep
  all 16 engines fed the whole span.
- Weights/x stream as bf16 (rel err ~2.5e-3 vs the 2e-2 gate), output in
  bf16 too: 14.45 (w) + 2.58 (x) + 0.8 (out) MB per core.
- Matmul: x window columns are the *stationary* operand (16-col LDWEIGHTS
  ~13 ns) and weight blocks are the *moving* operand at 1 col/cycle@2.4GHz.
  Contraction k = (r, c) on 96 partitions; q accumulates in PSUM. x is packed
  host-side so partition p = r*32+c holds x[b, c, row, 2v+r] (1.49x
  replication, vs 2.6x for the (q,c) packing).
- PSUM: out partitions = batch (16). Four PE column groups (tile_position
  (0, 32g)) pack v = 28h+7g+vl into one [128, 448] bank per (u, h). Each
  group needs its own start=True (the has_written clear only covers the
  partitions that matmul writes). vl-outer issue order lets the 4 column
  groups run concurrently in the array.
- Weight DMA in 28 per-(u,h,g) chunks (0.5MB each), bufs=16 so prefetch never
  stalls on tile recycling. DMA rings: weights alone on the SP HWDGE ring;
  x and outputs on the gpsimd SWDGE ring. Rationale: Tile assigns HWDGE
  completion-semaphore lanes round-robin across ALL HWDGE DMAs, and lane
  thresholds are only race-free if same-lane DMAs complete in tick order —
  guaranteed within one FIFO ring, NOT across the two HWDGE rings (observed
  as a rare NaN: an MM chunk consuming its weight tile ~5us before the DMA
  landed). SWDGE uses a separate lane class (DMASW), so it adds a second
  descriptor stream with no aliasing; it also keeps the out-DMAs (which
  wait on drains) from head-of-line-blocking weight prefetch.
  Bias is added on the host after the gather.
"""

import numpy as np

B = 16
C_IN = 32
C_OUT = 64
H_OUT = 56
W_OUT = 56
KK = 3
STRIDE = 2
H_IN = 113

N_CORES = 8
U_PER = H_OUT // N_CORES          # 7 output rows per core
ROWS_IN = (U_PER - 1) * STRIDE + KK  # 15 input rows per core
KPART = KK * C_IN                 # 96 contraction partitions (r, c)
G = 4                             # PE column groups
VL = 7                            # v per group per half
NH = 2                            # halves per u
XFREE = ROWS_IN * B * W_OUT       # x tile free elems (row, b, v) = 13440
WFREE_G = VL * KK * C_OUT         # weight free elems per (u, h, g) = 1344
OFREE_U = NH * VL * C_OUT         # output free elems per (u, g) = 896

_CACHE = {}


def _build():
    import concourse.mybir as mybir
    from concourse import bacc
    from concourse.tile import TileContext

    bf16 = mybir.dt.bfloat16
    f32 = mybir.dt.float32
    nc = bacc.Bacc("TRN2", target_bir_lowering=False, debug=False,
                   num_devices=N_CORES)
    # Host-prepacked tensors (see kernel()):
    #   xr[p, (row*16 + b)*56 + v] = x[b, c, 2u0+row, 2v+r],  p = r*32+c
    #   wr[u, h, p, (((g*7+vl)*3+q)*64 + o] = weights[u0+u, 28h+7g+vl, c, q, r, o]
    #   y[g, u, b, (h*7+vl)*64 + o] = out[b, o, u0+u, 28h+7g+vl] (no bias)
    xr_in = nc.dram_tensor("xr", [KPART, XFREE], bf16,
                           kind="ExternalInput").ap()
    wr_in = nc.dram_tensor("wr", [U_PER, NH, G, KPART, WFREE_G], bf16,
                           kind="ExternalInput").ap()
    y_out = nc.dram_tensor("y", [G, U_PER, B, OFREE_U], bf16,
                           kind="ExternalOutput").ap()

    with TileContext(nc) as tc:
        with tc.tile_pool(name="xpool", bufs=1) as xpool, \
             tc.tile_pool(name="wpool", bufs=16) as wpool, \
             tc.tile_pool(name="opool", bufs=1) as opool, \
             tc.tile_pool(name="pspool", bufs=6, space="PSUM") as pspool:

            # x in 5 row-chunks (3 input rows each) on the SWDGE ring;
            # chunk 0 unblocks u=0 after ~0.5MB, the rest stream behind it
            # well ahead of the u that needs them.
            xt = xpool.tile([KPART, XFREE], bf16)
            chunk = 3 * B * W_OUT
            for ci in range(5):
                nc.gpsimd.dma_start(out=xt[:, ci * chunk:(ci + 1) * chunk],
                                    in_=xr_in[:, ci * chunk:(ci + 1) * chunk])
            xt4 = xt.rearrange("p (row b v) -> p row b v", row=ROWS_IN, b=B)

            stage = opool.tile([128, U_PER * OFREE_U], bf16)
            st3 = stage.rearrange("p (u x) -> p u x", u=U_PER)

            for u in range(U_PER):
                for h in range(NH):
                    ps = pspool.tile([128, VL * C_OUT], f32)
                    ps3 = ps.rearrange("p (vl o) -> p vl o", vl=VL)
                    # weight sub-chunk per (u, h, g): ~0.5MB lands every
                    # ~2.2us, keeping the PE inside its ~3.4us HAM window
                    # (warm 2.4GHz) and the compute tail under 1us. Each
                    # col group gets its own start=True (the bank clear
                    # only covers the partitions that matmul writes).
                    for g in range(G):
                        wt = wpool.tile([KPART, WFREE_G], bf16)
                        nc.sync.dma_start(out=wt[:], in_=wr_in[u, h, g])
                        wt4 = wt.rearrange("p (vl q o) -> p vl q o",
                                           vl=VL, q=KK)
                        for vl in range(VL):
                            for q in range(KK):
                                lhsT = xt4[:, 2 * u + q, :, 28 * h + 7 * g + vl]
                                rhs = wt4[:, vl, q]               # [96, 64]
                                nc.tensor.matmul(
                                    ps3[32 * g:32 * g + 16, vl], lhsT, rhs,
                                    start=(vl == 0 and q == 0),
                                    stop=(vl == VL - 1 and q == KK - 1),
                                    tile_position=(0, 32 * g))
                    # whole-tile drain on the vector engine (f32->bf16);
                    # garbage partitions 16:32 etc. are copied, never DMA'd
                    nc.vector.tensor_scalar_add(
                        st3[:, u, 448 * h:448 * (h + 1)], ps[:, :], 0.0)
                # outs ride SWDGE (separate sem-lane class) except the last
                # u, which tails the sync ring right behind the final weight
                # chunk (same FIFO -> still race-free, shorter tail)
                oeng = nc.gpsimd if u < U_PER - 1 else nc.sync
                for g in range(G):
                    oeng.dma_start(out=y_out[g, u],
                                   in_=st3[32 * g:32 * g + 16, u])

    nc.compile()
    return nc


def _get_nc():
    if "nc" not in _CACHE:
        _CACHE["nc"] = _build()
    return _CACHE["nc"]


def kernel(x, weights, bias, _trace=False, _tmpdir=None):
    import ml_dtypes
    from concourse.bass_utils import run_bass_kernel_spmd

    bf16 = ml_dtypes.bfloat16
    x = np.asarray(x, dtype=np.float32)
    weights = np.asarray(weights, dtype=np.float32)
    bias = np.asarray(bias, dtype=np.float32)

    # wr: (core, u, h, g, p=(r,c), (vl,q,o))
    wb = weights.astype(bf16).reshape(N_CORES, U_PER, NH, G, VL,
                                      C_IN, KK, KK, C_OUT)
    wr_all = np.ascontiguousarray(
        wb.transpose(0, 1, 2, 3, 7, 5, 4, 6, 8)).reshape(
            N_CORES, U_PER, NH, G, KPART, WFREE_G)

    xb = x.astype(bf16)
    in_maps = []
    for i in range(N_CORES):
        u0 = i * U_PER
        xs = xb[:, :, STRIDE * u0:STRIDE * u0 + ROWS_IN, :]  # (B, C, 15, 113)
        # (r, c, row, b, v): p = r*32+c holds x[b, c, row, 2v+r]
        xq = np.stack([xs[:, :, :, r::2][:, :, :, :W_OUT] for r in range(KK)],
                      axis=0)                                # (r, B, C, 15, 56)
        xr = np.ascontiguousarray(xq.transpose(0, 2, 3, 1, 4)).reshape(
            KPART, XFREE)
        in_maps.append({"xr": xr, "wr": wr_all[i]})

    nc = _get_nc()
    core_ids = list(range(N_CORES))
    res = run_bass_kernel_spmd(nc, in_maps, core_ids, trace=_trace,
                               tmpdir=_tmpdir)
    # y per core: (G, U_PER, B, (h, vl, o)) -> (b, o, core*7+u, 28h+7g+vl)
    ys = np.stack([np.asarray(res.results[i]["y"]) for i in core_ids])
    ys = ys.reshape(N_CORES, G, U_PER, B, NH, VL, C_OUT).astype(np.float32)
    out = np.ascontiguousarray(
        ys.transpose(3, 6, 0, 2, 4, 1, 5)).reshape(B, C_OUT, H_OUT, W_OUT)
    out += bias.transpose(2, 0, 1)[None]
    if _trace:
        _CACHE["last_result"] = res
    return out
